# revision 13
# baseline (speedup 1.0000x reference)
"""DenseCapsule dynamic-routing kernel for 8 Trainium2 NeuronCores.

v2 strategy (contraction/n sharding, full batch per core):
  - s0 (uniform-c iteration) is computed REDUNDANTLY on every core from the
    full (replicated) x and W2 streams -> no AllReduce on the critical path
    until mid-kernel. A warmup AllReduce issued at t=0 absorbs the one-time
    collective barrier (~50us) off the critical path.
  - The single mid-kernel AllReduce of s1 is split into 3 o-group blocks
    pipelined with the y*s compute; each block feeds its own g_chain ->
    agreement pipeline immediately on arrival.
  - Routing algebra as v1: x_hat never materialized; all contractions run
    through W2 on the PE. Elementwise work (t~*x, c*x, softmax) is spread
    across DVE / ACT / Pool with bf16 2x-mode APs where possible.
"""

import sys

sys.path.insert(0, "/opt/trn_rl_repo")

import numpy as np
import ml_dtypes

import concourse.bass as bass  # noqa: F401
import concourse.tile as tile
from concourse import bacc, mybir
from concourse.bass_utils import run_bass_kernel_spmd

B, N_IN, D_IN, N_OUT, D_OUT = 512, 1152, 8, 10, 16
NCORES = 8
NLOC = N_IN // NCORES  # 144
F = NLOC * D_IN        # 1152 f-rows per core, f = 8*n_within + j
NCH = F // 128         # 9 chunks
FALL = N_IN * D_IN     # 9216 f-rows total
NCH_ALL = FALL // 128  # 72
NREST = NCH_ALL - NCH  # 63 streamed chunks
OI = N_OUT * D_OUT     # 160
BF16 = mybir.dt.bfloat16
F32 = mybir.dt.float32
AF = mybir.ActivationFunctionType
ALU = mybir.AluOpType
bfnp = ml_dtypes.bfloat16

GROUPS = ((0, 4), (1, 4), (2, 2))  # (g, nu): o = 4*g + u

_built = None


def _build():
    nc = bacc.Bacc("TRN2", target_bir_lowering=False, debug=False, num_devices=NCORES)

    xT_d = nc.dram_tensor("xT", [F, B], BF16, kind="ExternalInput")
    w2_d = nc.dram_tensor("w2", [F, OI], BF16, kind="ExternalInput")
    xrest_d = nc.dram_tensor("xrest", [128, NREST * B], BF16, kind="ExternalInput")
    wrest_d = nc.dram_tensor("wrest", [128, NREST * OI], BF16, kind="ExternalInput")
    w2t_d = nc.dram_tensor("w2t", [384, F], BF16, kind="ExternalInput")
    w2p_d = nc.dram_tensor("w2p", [F, 320], BF16, kind="ExternalInput")
    bd_d = nc.dram_tensor("bd", [128, 8 * 128], BF16, kind="ExternalInput")
    osel_d = nc.dram_tensor("osel", [384, 16], BF16, kind="ExternalInput")
    ident_d = nc.dram_tensor("ident", [128, 128], BF16, kind="ExternalInput")
    out_d = nc.dram_tensor("out", [OI, B], BF16, kind="ExternalOutput")

    with tile.TileContext(nc) as tc, nc.allow_low_precision(
            reason="bf16 softmax/routing logits are within tolerance"):
        _emit(tc, nc, xT_d, w2_d, xrest_d, wrest_d, w2t_d, w2p_d, bd_d, osel_d, ident_d, out_d)
    nc.compile()
    return nc


def _emit(tc, nc, xT_d, w2_d, xrest_d, wrest_d, w2t_d, w2p_d, bd_d, osel_d, ident_d, out_d):
    from contextlib import ExitStack

    ctx = ExitStack()
    const = ctx.enter_context(tc.tile_pool(name="const", bufs=1))
    small = ctx.enter_context(tc.tile_pool(name="small", bufs=1))
    sp = ctx.enter_context(tc.tile_pool(name="sp", bufs=2))
    cxp = ctx.enter_context(tc.tile_pool(name="cx", bufs=2))
    yp = ctx.enter_context(tc.tile_pool(name="y", bufs=2))
    pp = ctx.enter_context(tc.tile_pool(name="p", bufs=6))
    tsbp = ctx.enter_context(tc.tile_pool(name="tsb", bufs=6))
    gp = ctx.enter_context(tc.tile_pool(name="gp", bufs=2))
    psp = ctx.enter_context(tc.tile_pool(name="psp", bufs=8, space="PSUM"))
    dram = ctx.enter_context(tc.tile_pool(name="dram", bufs=1, space="DRAM"))

    # ---- collective warmup: absorbs the one-time CC barrier off-path ----
    wu_in = dram.tile([16, 16], F32, tag="wu_in", name="wu_in")
    wu_out = dram.tile([16, 16], F32, tag="wu_out", name="wu_out")
    nc.gpsimd.collective_compute(
        "AllReduce", ALU.add, replica_groups=[list(range(NCORES))],
        ins=[wu_in.opt()], outs=[wu_out.opt()],
    )

    # ---- load constants (ordered by first use) ----
    xT = []
    for c in range(NCH):
        t = const.tile([128, B], BF16, tag=f"xT{c}", name=f"xT{c}")
        (nc.sync if c % 2 else nc.scalar).dma_start(t[:], xT_d[128 * c:128 * (c + 1), :])
        xT.append(t)
    w2l = []
    for c in range(NCH):
        t = const.tile([128, OI], BF16, tag=f"w2l{c}", name=f"w2l{c}")
        (nc.scalar if c % 2 else nc.sync).dma_start(t[:], w2_d[128 * c:128 * (c + 1), :])
        w2l.append(t)
    w2tp = []
    oselg = []
    for g in range(3):
        t = const.tile([128, F], BF16, tag=f"w2tp{g}", name=f"w2tp{g}")
        (nc.sync if g % 2 else nc.scalar).dma_start(t[:], w2t_d[128 * g:128 * (g + 1), :])
        w2tp.append(t)
        t2 = const.tile([128, 16], BF16, tag=f"oselg{g}", name=f"oselg{g}")
        nc.sync.dma_start(t2[:], osel_d[128 * g:128 * (g + 1), :])
        oselg.append(t2)
    bd = const.tile([128, 8 * 128], BF16, tag="bd", name="bd")
    nc.scalar.dma_start(bd[:], bd_d[:])
    ident = const.tile([128, 128], BF16, tag="ident", name="ident")
    nc.sync.dma_start(ident[:], ident_d[:])

    # streamed x/w2 for the replicated s0 (8 super-groups, doublebuffered)
    SG = (8, 8, 8, 8, 8, 8, 8, 7)
    sg_off = [0, 8, 16, 24, 32, 40, 48, 56]

    w2p = []
    for c in range(NCH):
        t = const.tile([128, 320], BF16, tag=f"w2p{c}", name=f"w2p{c}")
        (nc.sync if c % 2 else nc.scalar).dma_start(t[:], w2p_d[128 * c:128 * (c + 1), :])
        w2p.append(t)

    # ---- persistent per-routing tiles ----
    OB = N_OUT * B  # 5120
    s_red3 = []
    sTg3 = []
    grep3 = []
    sq3 = []
    s_part3 = []
    for g in range(3):
        r = small.tile([128, B], BF16, tag=f"sred3{g}", name=f"sred3{g}")
        nc.vector.memset(r[:], 0.0)
        s_red3.append(r)
        r = small.tile([128, B], BF16, tag=f"sTg3{g}", name=f"sTg3{g}")
        sTg3.append(r)
        r = small.tile([128, B], BF16, tag=f"grep3{g}", name=f"grep3{g}")
        nc.vector.memset(r[:], 0.0)
        grep3.append(r)
        r = small.tile([128, B], BF16, tag=f"sq3{g}", name=f"sq3{g}")
        sq3.append(r)
        r = small.tile([128, B], BF16, tag=f"spart3{g}", name=f"spart3{g}")
        s_part3.append(r)
    state_a = small.tile([128, OB], BF16, tag="sta", name="sta")
    state_b = small.tile([16, OB], BF16, tag="stb", name="stb")
    e_a = small.tile([128, OB], BF16, tag="e_a", name="e_a")
    e_b = small.tile([16, OB], BF16, tag="e_b", name="e_b")

    ar_in = dram.tile([OI, B], BF16, tag="arin", name="arin")
    ar_out = dram.tile([OI, B], BF16, tag="arout", name="arout")
    c_dram = [dram.tile([NLOC, OB], BF16, tag="cdram", name=f"cdram{t}", bufs=2)
              for t in range(2)]

    def sl(o):
        return slice(B * o, B * (o + 1))

    # ================= replicated s0 (uniform c) ==========================
    p0a = psp.tile([128, B], F32, tag="ps", name="s0a")
    p0b = psp.tile([32, B], F32, tag="ps", name="s0b")
    for c in range(NCH):
        nc.tensor.matmul(p0a[:], w2l[c][:, 0:128], xT[c][:],
                         start=(c == 0), stop=False)
        nc.tensor.matmul(p0b[:], w2l[c][:, 128:160], xT[c][:],
                         start=(c == 0), stop=False)
    dma_engs = (nc.sync, nc.scalar)
    for G in range(8):
        ng = SG[G]
        xs = sp.tile([128, 8 * B], BF16, tag="xs", name=f"xs{G}")
        ws = sp.tile([128, 8 * OI], BF16, tag="ws", name=f"ws{G}")
        dma_engs[G % 2].dma_start(
            xs[:, 0:ng * B], xrest_d[:, sg_off[G] * B:(sg_off[G] + ng) * B])
        nc.gpsimd.dma_start(
            ws[:, 0:ng * OI], wrest_d[:, sg_off[G] * OI:(sg_off[G] + ng) * OI])
        for i in range(ng):
            last = (G == 7 and i == ng - 1)
            nc.tensor.matmul(p0a[:], ws[:, OI * i:OI * i + 128],
                             xs[:, B * i:B * (i + 1)],
                             start=False, stop=last)
            nc.tensor.matmul(p0b[:], ws[:, OI * i + 128:OI * (i + 1)],
                             xs[:, B * i:B * (i + 1)],
                             start=False, stop=last)
    # scatter s0 psum -> sbuf -> s_red3 bands (psum reads must be 32-aligned)
    s0sb_a = small.tile([128, B], BF16, tag="s0sba", name="s0sba")
    s0sb_b = small.tile([32, B], BF16, tag="s0sbb", name="s0sbb")
    nc.scalar.copy(s0sb_a[:], p0a[:])
    nc.vector.tensor_copy(s0sb_b[:], p0b[:])
    for o in range(N_OUT):
        g, u = o // 4, o % 4
        src = s0sb_a[16 * o:16 * (o + 1), :] if o < 8 else s0sb_b[16 * (o - 8):16 * (o - 7), :]
        (nc.sync if o % 2 else nc.scalar).dma_start(
            s_red3[g][32 * u:32 * u + 16, :], src)

    # ---------------- helpers --------------------------------------------
    def g_chain_g(t, g, nu, alpha):
        """per-group squash gain: grep3[g] rows <- ghat; sTg3[g] <- ghat*s."""
        pn2 = psp.tile([16, B], F32, tag="ps", name=f"n2_{t}{g}")
        nc.gpsimd.tensor_mul(sq3[g][:], s_red3[g][:], s_red3[g][:])
        nc.tensor.matmul(pn2[:], oselg[g][:], sq3[g][:], start=True, stop=True)
        a2 = float(alpha * alpha)
        g_ln = gp.tile([16, B], F32, tag="gln", name=f"gln{t}{g}")
        nc.scalar.activation(g_ln[:], pn2[:], AF.Ln, scale=a2)
        g_rt = gp.tile([16, B], F32, tag="grt", name=f"grt{t}{g}")
        nc.scalar.activation(g_rt[:], g_ln[:], AF.Exp, scale=0.5)
        g_d = gp.tile([16, B], F32, tag="gd", name=f"gd{t}{g}")
        nc.vector.tensor_scalar(g_d[:], pn2[:], float(alpha), 1.0 / float(alpha),
                                ALU.mult, ALU.add)
        g_r = gp.tile([16, B], F32, tag="gr", name=f"gr{t}{g}")
        nc.vector.reciprocal_approx_fast(g_r[:], g_d[:])
        g_hat = gp.tile([16, B], BF16, tag="ghat", name=f"ghat{t}{g}")
        nc.vector.tensor_mul(g_hat[:], g_rt[:], g_r[:])
        # replicate ghat rows (o on rows) to 16-row bands via DRAM bounce
        gd = dram.tile([16, B], BF16, tag="gdram", name=f"gd{t}{g}", bufs=2)
        nc.scalar.dma_start(gd[:], g_hat[:])
        for u in range(nu):
            o = 4 * g + u
            nc.scalar.dma_start(
                grep3[g][32 * u:32 * u + 16, :],
                gd[o:o + 1, :].broadcast_to((16, B)),
            )
        nc.vector.tensor_mul(sTg3[g][:], grep3[g][:], s_red3[g][:])

    zseed = {}

    def agreement_g(t, g, nu):
        """state[t] o-slices and e=exp(state) for o in group g."""
        for u in range(nu):
            o = 4 * g + u
            pba = psp.tile([128, B], F32, tag="ps", name=f"ba{t}{o}")
            pbb = psp.tile([16, B], F32, tag="ps", name=f"bb{t}{o}")
            if t == 1:
                # fold prior b-logit state into the PSUM accumulation (PE)
                nc.tensor.matmul(pba[:], ident[:], state_a[:, sl(o)],
                                 start=True, stop=False)
                nc.tensor.matmul(pbb[:], ident[0:16, 0:16], state_b[:, sl(o)],
                                 start=True, stop=False)
            for c in range(NCH):
                pt = psp.tile([128, B], F32, tag="ps", name=f"t{t}{o}{c}")
                nc.tensor.matmul(
                    pt[:], w2tp[g][32 * u:32 * (u + 1), 128 * c:128 * (c + 1)],
                    sTg3[g][32 * u:32 * (u + 1), :],
                    start=True, stop=True, tile_position=(32 * u, 0))
                p = pp.tile([128, B], BF16, tag="p", name="p")
                if c % 2 == 0 or c == 7:
                    # ACT copy (psum->sbuf bf16) + Pool/DVE 2x mul
                    tsb = tsbp.tile([128, B], BF16, tag="tsb", name="tsb")
                    nc.scalar.copy(tsb[:], pt[:])
                    if c in (0, 2, 4):
                        nc.gpsimd.tensor_mul(p[:], tsb[:], xT[c][:])
                    else:
                        nc.vector.tensor_mul(p[:], tsb[:], xT[c][:])
                else:
                    # DVE direct-psum mul (1x)
                    nc.vector.tensor_mul(p[:], pt[:], xT[c][:])
                if c < 8:
                    nc.tensor.matmul(pba[:], bd[:, 128 * c:128 * (c + 1)],
                                     p[:], start=(t == 0 and c == 0), stop=(c == 7))
                else:
                    nc.tensor.matmul(pbb[:], bd[:, 0:16], p[:],
                                     start=(t == 0), stop=True)
            # state / exp per o (b-logits now complete in PSUM)
            nc.scalar.activation(e_a[:, sl(o)], pba[:], AF.Exp)
            nc.scalar.activation(e_b[:, sl(o)], pbb[:], AF.Exp)
            if t == 0:
                nc.vector.tensor_copy(state_a[:, sl(o)], pba[:])
                nc.vector.tensor_copy(state_b[:, sl(o)], pbb[:])
            # accumulate softmax denominator
            za, zb = zseed[t]
            if o == 0:
                nc.vector.tensor_copy(za[:], e_a[:, sl(o)])
                nc.gpsimd.tensor_copy(zb[:], e_b[:, sl(o)])
            else:
                nc.vector.tensor_add(za[:], za[:], e_a[:, sl(o)])
                nc.gpsimd.tensor_add(zb[:], zb[:], e_b[:, sl(o)])

    def softmax_tail(t):
        """normalize e_a/e_b in place by 1/z (approx reciprocal)."""
        za, zb = zseed[t]
        zaf = small.tile([128, B], F32, tag=f"zaf{t}", name=f"zaf{t}")
        zbf = small.tile([16, B], F32, tag=f"zbf{t}", name=f"zbf{t}")
        nc.vector.tensor_copy(zaf[:], za[:])
        nc.gpsimd.tensor_copy(zbf[:], zb[:])
        zira = small.tile([128, B], F32, tag=f"zira{t}", name=f"zira{t}")
        zirb = small.tile([16, B], F32, tag=f"zirb{t}", name=f"zirb{t}")
        nc.vector.reciprocal_approx_fast(zira[:], zaf[:])
        nc.vector.reciprocal_approx_fast(zirb[:], zbf[:])
        zi_a = small.tile([128, B], BF16, tag=f"zia{t}", name=f"zia{t}")
        zi_b = small.tile([16, B], BF16, tag=f"zib{t}", name=f"zib{t}")
        nc.vector.tensor_copy(zi_a[:], zira[:])
        nc.gpsimd.tensor_copy(zi_b[:], zirb[:])
        ea3 = e_a[:].rearrange("p (o b) -> p o b", o=N_OUT)
        eb3 = e_b[:].rearrange("p (o b) -> p o b", o=N_OUT)
        nc.vector.tensor_mul(
            ea3, ea3, zi_a[:].unsqueeze(1).broadcast_to((128, N_OUT, B)))
        nc.gpsimd.tensor_mul(
            eb3, eb3, zi_b[:].unsqueeze(1).broadcast_to((16, N_OUT, B)))
        nc.sync.dma_start(c_dram[t][0:128, :], e_a[:])
        nc.scalar.dma_start(c_dram[t][128:NLOC, :], e_b[:])

    def y_s_group(itn, g, nu):
        """s_part3[g] rows 32u:+16 <- sum_f W2[f,(o,:)] * (c (.) x)[f,:]."""
        w0 = 4 * g
        cd = c_dram[itn - 1]
        psos = psp.tile([128, B], F32, tag="ps", name=f"so{itn}{g}")
        for c in range(NCH):
            cx = cxp.tile([128, 4 * B], BF16, tag="cx", name="cx")
            (nc.sync if c % 2 else nc.scalar).dma_start(
                cx[:, 0:nu * B],
                cd[16 * c:16 * (c + 1),
                   B * w0:B * (w0 + nu)].unsqueeze(1).broadcast_to(
                    (16, 8, nu * B)),
            )
            y = yp.tile([128, 4 * B], BF16, tag="y", name="y")
            xbc = xT[c][:].unsqueeze(1).broadcast_to((128, nu, B))
            ybc = y[:, 0:nu * B].rearrange("p (o b) -> p o b", o=nu)
            cbc = cx[:, 0:nu * B].rearrange("p (o b) -> p o b", o=nu)
            if c % 3 == 2:
                nc.gpsimd.tensor_mul(ybc, cbc, xbc)
            else:
                nc.vector.tensor_mul(ybc, cbc, xbc)
            for u in range(nu):
                o = w0 + u
                nc.tensor.matmul(psos[32 * u:32 * (u + 1), :],
                                 w2p[c][:, 32 * o:32 * (o + 1)],
                                 y[:, B * u:B * (u + 1)],
                                 start=(c == 0), stop=(c == NCH - 1),
                                 tile_position=(0, 32 * u))
        nc.scalar.copy(s_part3[g][:], psos[:])

    # =====================  routing  =====================================
    # ---- iteration 0 (s0 replicated; all groups ready) ----
    zseed[0] = (small.tile([128, B], BF16, tag="za0", name="za0"),
                small.tile([16, B], BF16, tag="zb0", name="zb0"))
    for (g, nu) in GROUPS:
        g_chain_g(0, g, nu, 0.1)
        agreement_g(0, g, nu)
    softmax_tail(0)
    for (g, nu) in GROUPS:
        y_s_group(1, g, nu)
        # ship this block's partial s1 to the collective
        for u in range(nu):
            o = 4 * g + u
            nc.sync.dma_start(ar_in[16 * o:16 * (o + 1), :],
                              s_part3[g][32 * u:32 * u + 16, :])
        nc.gpsimd.collective_compute(
            "AllReduce", ALU.add, replica_groups=[list(range(NCORES))],
            ins=[ar_in[64 * g:64 * g + 16 * nu, :].opt()],
            outs=[ar_out[64 * g:64 * g + 16 * nu, :].opt()],
        )

    # ---- iteration 1 (blocks arrive in order) ----
    zseed[1] = (small.tile([128, B], BF16, tag="za1", name="za1"),
                small.tile([16, B], BF16, tag="zb1", name="zb1"))
    for (g, nu) in GROUPS:
        for u in range(nu):
            o = 4 * g + u
            nc.sync.dma_start(s_red3[g][32 * u:32 * u + 16, :],
                              ar_out[16 * o:16 * (o + 1), :])
        g_chain_g(1, g, nu, 1.0)
        agreement_g(1, g, nu)
    softmax_tail(1)
    for (g, nu) in GROUPS:
        y_s_group(2, g, nu)
        for u in range(nu):
            o = 4 * g + u
            nc.sync.dma_start(out_d[16 * o:16 * (o + 1), :],
                              s_part3[g][32 * u:32 * u + 16, :])

    ctx.close()


def _prep_inputs(x, weight):
    """Host-side layout prep. Returns per-core input maps."""
    x = np.asarray(x, dtype=np.float32)
    weight = np.asarray(weight, dtype=np.float32)
    bd_all = np.zeros((128, 8 * 128), dtype=bfnp)
    for cp in range(8):
        for p in range(128):
            bd_all[p, 128 * cp + 16 * cp + p // 8] = 1.0
    # oselg: [3][128, 16]; row p = 32u + i (i<16 live), col m = o = 4g+u
    oselg = np.zeros((3, 128, 16), dtype=bfnp)
    for g in range(3):
        for u in range(4 if g < 2 else 2):
            oselg[g, 32 * u:32 * u + 16, 4 * g + u] = 1.0
    oselg = oselg.reshape(384, 16)
    ident = np.eye(128, dtype=bfnp)

    # full-x / full-w2 chunk views (for the replicated s0)
    xT_all = np.ascontiguousarray(
        x.transpose(1, 2, 0).reshape(FALL, B)).astype(bfnp)      # [9216, B]
    w2_all = np.ascontiguousarray(
        weight.transpose(1, 3, 0, 2).reshape(FALL, OI)).astype(bfnp)  # [9216, OI]

    in_maps = []
    for k in range(NCORES):
        n0 = NLOC * k
        f0 = n0 * D_IN
        xT = xT_all[f0:f0 + F]                   # [1152, B] local
        w2 = w2_all[f0:f0 + F]                   # [1152, OI] local
        # rest chunks (other cores' rows), laid out column-blocked [128, 63*B]
        rest_rows = np.concatenate([xT_all[:f0], xT_all[f0 + F:]], axis=0)
        rest_w = np.concatenate([w2_all[:f0], w2_all[f0 + F:]], axis=0)
        xrest = np.ascontiguousarray(
            rest_rows.reshape(NREST, 128, B).transpose(1, 0, 2).reshape(128, NREST * B))
        wrest = np.ascontiguousarray(
            rest_w.reshape(NREST, 128, OI).transpose(1, 0, 2).reshape(128, NREST * OI))
        w2t = np.ascontiguousarray(w2.T)          # [160, F]
        # w2tp: [3][128, F], rows 32u+0:16 = w2t rows of o=4g+u, rest zero
        w2tp = np.zeros((3, 128, F), dtype=bfnp)
        for g in range(3):
            for u in range(4 if g < 2 else 2):
                o = 4 * g + u
                w2tp[g, 32 * u:32 * u + 16, :] = w2t[16 * o:16 * (o + 1), :]
        w2tp = w2tp.reshape(384, F)
        # w2p: [F, 320], cols 32o+i (i<16) = w2 col 16o+i, rest zero
        w2p = np.zeros((F, 320), dtype=bfnp)
        for o in range(N_OUT):
            w2p[:, 32 * o:32 * o + 16] = w2[:, 16 * o:16 * (o + 1)]
        in_maps.append({
            "xT": xT, "w2": w2, "xrest": xrest, "wrest": wrest,
            "w2t": w2tp, "w2p": w2p, "bd": bd_all, "osel": oselg,
            "ident": ident,
        })
    return in_maps


def _squash_np(s):
    norm = np.linalg.norm(s, axis=-1, keepdims=True)
    return (norm ** 2 / (1.0 + norm ** 2) / (norm + 1e-8)) * s


def run_spmd(x, weight, trace=False, tmpdir=None):
    global _built
    if _built is None:
        _built = _build()
    nc = _built
    in_maps = _prep_inputs(x, weight)
    res = run_bass_kernel_spmd(
        nc, in_maps, list(range(NCORES)), trace=trace, tmpdir=tmpdir)
    s2 = np.zeros((OI, B), dtype=np.float32)
    for k in range(NCORES):
        s2 += res.results[k]["out"].astype(np.float32)
    s2 = s2.reshape(N_OUT, D_OUT, B).transpose(2, 0, 1)  # [B, 10, 16]
    out = _squash_np(s2).astype(np.float32)
    return out, res


def kernel(x, weight):
    out, _ = run_spmd(x, weight)
    return out


# revision 15
# speedup vs baseline: 1.2139x; 1.2139x over previous
"""DenseCapsule dynamic-routing kernel for 8 Trainium2 NeuronCores.

v3 strategy (contraction/n sharding, full batch per core):
  - All routing contractions run through the shared weight W on the PE;
    x_hat is never materialized (see v1 docstring for the algebra).
  - Both AllReduces (s0, s1) are split into 3 o-group blocks; each block
    feeds its own g_chain -> agreement pipeline on arrival, overlapping
    collective latency with compute. No warmup collective: the first AR
    block absorbs the one-time CC-relay boot (~70us) itself.
  - b-logit accumulation across iterations happens on the PE via an
    identity matmul folded into the PSUM accumulation group (no DVE adds).
  - Activation-table thrash is eliminated by biasing the act-func-set
    chooser toward the combined ln+exp table.
  - Engine policy: PSUM reads only on ACT/DVE (hardware restriction);
    Pool (gpsimd) runs the 16-row n-leftover sidecar chain off the
    critical path; softmax reciprocal via the fast DVE approximation.
"""

import sys

sys.path.insert(0, "/opt/trn_rl_repo")

import numpy as np
import ml_dtypes

import concourse.bass as bass  # noqa: F401
import concourse.tile as tile
from concourse import bacc, mybir
from concourse.bass_utils import run_bass_kernel_spmd

B, N_IN, D_IN, N_OUT, D_OUT = 512, 1152, 8, 10, 16
NCORES = 8
NLOC = N_IN // NCORES  # 144
F = NLOC * D_IN        # 1152 f-rows per core, f = 8*n_within + j
NCH = F // 128         # 9 chunks
OI = N_OUT * D_OUT     # 160
BF16 = mybir.dt.bfloat16
F32 = mybir.dt.float32
AF = mybir.ActivationFunctionType
ALU = mybir.AluOpType
bfnp = ml_dtypes.bfloat16

GROUPS = ((0, 4), (1, 4), (2, 2))  # (g, nu): o = 4*g + u

_built = None


def _patch_act_tables():
    """Bias the act-func-set chooser so ln and exp resolve to the single
    combined table (ids stay canonical; only membership used for choosing
    is masked), avoiding per-phase ACT table reloads."""
    import functools
    import concourse.hw_specs as hw_specs
    import concourse.bacc as bacc_mod

    orig = hw_specs.get_activation_tables.__wrapped__

    @functools.cache
    def patched(module_arch):
        tabs = dict(orig(module_arch))
        out = {}
        for name, funcs in tabs.items():
            funcs = set(funcs)
            if name != "natural_log_exp_and_others":
                funcs.discard(mybir.ActivationFunctionType.Exp)
                funcs.discard(mybir.ActivationFunctionType.Ln)
            out[name] = funcs
        return out

    hw_specs.get_activation_tables = patched
    bacc_mod.get_activation_tables = patched


def _build():
    _patch_act_tables()
    nc = bacc.Bacc("TRN2", target_bir_lowering=False, debug=False, num_devices=NCORES)

    xT_d = nc.dram_tensor("xT", [F, B], BF16, kind="ExternalInput")
    w2_d = nc.dram_tensor("w2", [F, OI], BF16, kind="ExternalInput")
    w2t_d = nc.dram_tensor("w2t", [384, F], BF16, kind="ExternalInput")
    w2p_d = nc.dram_tensor("w2p", [F, 320], BF16, kind="ExternalInput")
    bd_d = nc.dram_tensor("bd", [128, 8 * 128], BF16, kind="ExternalInput")
    osel_d = nc.dram_tensor("osel", [384, 16], BF16, kind="ExternalInput")
    ident_d = nc.dram_tensor("ident", [128, 128], BF16, kind="ExternalInput")
    out_d = nc.dram_tensor("out", [OI, B], BF16, kind="ExternalOutput")

    with tile.TileContext(nc) as tc, nc.allow_low_precision(
            reason="bf16 softmax/routing logits are within tolerance"):
        _emit(tc, nc, xT_d, w2_d, w2t_d, w2p_d, bd_d, osel_d, ident_d, out_d)
    nc.compile()
    return nc


def _emit(tc, nc, xT_d, w2_d, w2t_d, w2p_d, bd_d, osel_d, ident_d, out_d):
    from contextlib import ExitStack

    ctx = ExitStack()
    const = ctx.enter_context(tc.tile_pool(name="const", bufs=1))
    small = ctx.enter_context(tc.tile_pool(name="small", bufs=1))
    gp = ctx.enter_context(tc.tile_pool(name="gp", bufs=2))
    cxp = ctx.enter_context(tc.tile_pool(name="cx", bufs=3))
    yp = ctx.enter_context(tc.tile_pool(name="y", bufs=3))
    pp = ctx.enter_context(tc.tile_pool(name="p", bufs=6))
    tsbp = ctx.enter_context(tc.tile_pool(name="tsb", bufs=6))
    psp = ctx.enter_context(tc.tile_pool(name="psp", bufs=8, space="PSUM"))
    dram = ctx.enter_context(tc.tile_pool(name="dram", bufs=1, space="DRAM"))

    # ---- load constants (ordered by first use) ----
    xTb = const.tile([128, NCH * B], BF16, tag="xTb", name="xTb")
    xTb3 = xTb[:].rearrange("p (c b) -> p c b", c=NCH)
    xTd3 = xT_d[:].rearrange("(c p) b -> p c b", p=128)
    for h in range(3):
        (nc.sync, nc.scalar, nc.sync)[h].dma_start(
            xTb3[:, 3 * h:3 * (h + 1), :], xTd3[:, 3 * h:3 * (h + 1), :])

    def xT(c):
        return xTb[:, c * B:(c + 1) * B]

    w2l = []
    for c in range(NCH):
        t = const.tile([128, OI], BF16, tag=f"w2l{c}", name=f"w2l{c}")
        (nc.scalar if c % 2 else nc.sync).dma_start(t[:], w2_d[128 * c:128 * (c + 1), :])
        w2l.append(t)
    w2tp = []
    oselg = []
    for g in range(3):
        t = const.tile([128, F], BF16, tag=f"w2tp{g}", name=f"w2tp{g}")
        (nc.sync if g % 2 else nc.scalar).dma_start(t[:], w2t_d[128 * g:128 * (g + 1), :])
        w2tp.append(t)
        t2 = const.tile([128, 16], BF16, tag=f"oselg{g}", name=f"oselg{g}")
        nc.sync.dma_start(t2[:], osel_d[128 * g:128 * (g + 1), :])
        oselg.append(t2)
    bd = const.tile([128, 8 * 128], BF16, tag="bd", name="bd")
    nc.scalar.dma_start(bd[:], bd_d[:])
    ident = const.tile([128, 128], BF16, tag="ident", name="ident")
    nc.sync.dma_start(ident[:], ident_d[:])
    w2p = []
    for c in range(NCH):
        t = const.tile([128, 320], BF16, tag=f"w2p{c}", name=f"w2p{c}")
        (nc.sync if c % 2 else nc.scalar).dma_start(t[:], w2p_d[128 * c:128 * (c + 1), :])
        w2p.append(t)

    # ---- persistent per-routing tiles ----
    OB = N_OUT * B  # 5120
    s_red3 = []
    sTg3 = []
    grep3 = []
    sq3 = []
    s_part3 = []
    for g in range(3):
        r = small.tile([128, B], BF16, tag=f"sred3{g}", name=f"sred3{g}")
        nc.vector.memset(r[:], 0.0)
        s_red3.append(r)
        r = small.tile([128, B], BF16, tag=f"sTg3{g}", name=f"sTg3{g}")
        sTg3.append(r)
        r = small.tile([128, B], BF16, tag=f"grep3{g}", name=f"grep3{g}")
        nc.vector.memset(r[:], 0.0)
        grep3.append(r)
        r = small.tile([128, B], BF16, tag=f"sq3{g}", name=f"sq3{g}")
        sq3.append(r)
        r = small.tile([128, B], BF16, tag=f"spart3{g}", name=f"spart3{g}")
        s_part3.append(r)
    state_a = small.tile([128, OB], BF16, tag="sta", name="sta")
    state_b = small.tile([16, OB], BF16, tag="stb", name="stb")
    e_a = small.tile([128, OB], BF16, tag="e_a", name="e_a")
    e_b = small.tile([16, OB], BF16, tag="e_b", name="e_b")

    ar_in = [dram.tile([OI, B], BF16, tag=f"arin{t}", name=f"arin{t}") for t in range(2)]
    ar_out = [dram.tile([OI, B], BF16, tag=f"arout{t}", name=f"arout{t}") for t in range(2)]
    c_dram = [dram.tile([NLOC, OB], BF16, tag=f"cdram{t}", name=f"cdram{t}")
              for t in range(2)]

    def sl(o):
        return slice(B * o, B * (o + 1))

    def ship_block(ar, g, nu, src):
        """DMA an o-group block of s rows to DRAM and AllReduce it."""
        for u in range(nu):
            o = 4 * g + u
            nc.sync.dma_start(ar_in[ar][16 * o:16 * (o + 1), :],
                              src[32 * u:32 * u + 16, :])
        nc.gpsimd.collective_compute(
            "AllReduce", ALU.add, replica_groups=[list(range(NCORES))],
            ins=[ar_in[ar][64 * g:64 * g + 16 * nu, :].opt()],
            outs=[ar_out[ar][64 * g:64 * g + 16 * nu, :].opt()],
        )

    # ================= s0 partial (uniform c) + split AllReduce ===========
    p0a = psp.tile([128, B], F32, tag="ps", name="s0a")
    p0b = psp.tile([32, B], F32, tag="ps", name="s0b")
    for c in range(NCH):
        nc.tensor.matmul(p0a[:], w2l[c][:, 0:128], xT(c),
                         start=(c == 0), stop=(c == NCH - 1))
        nc.tensor.matmul(p0b[:], w2l[c][:, 128:160], xT(c),
                         start=(c == 0), stop=(c == NCH - 1))
    s0sb_a = small.tile([128, B], BF16, tag="s0sba", name="s0sba")
    s0sb_b = small.tile([32, B], BF16, tag="s0sbb", name="s0sbb")
    nc.scalar.copy(s0sb_a[:], p0a[:])
    nc.vector.tensor_copy(s0sb_b[:], p0b[:])
    # band-layout staging so ship_block's source indexing is uniform
    s0st = small.tile([128, 3 * B], BF16, tag="s0st", name="s0st")
    for o in range(N_OUT):
        g, u = o // 4, o % 4
        src = s0sb_a[16 * o:16 * (o + 1), :] if o < 8 else \
            s0sb_b[16 * (o - 8):16 * (o - 7), :]
        (nc.sync if o % 2 else nc.scalar).dma_start(
            s0st[32 * u:32 * u + 16, g * B:(g + 1) * B], src)
    for (g, nu) in GROUPS:
        ship_block(0, g, nu, s0st[:, g * B:(g + 1) * B])

    # ---------------- helpers --------------------------------------------
    def g_chain_g(t, g, nu, alpha):
        """per-group squash gain: grep3[g] rows <- ghat; sTg3[g] <- ghat*s."""
        pn2 = psp.tile([16, B], F32, tag="ps", name=f"n2_{t}{g}")
        nc.vector.tensor_mul(sq3[g][:], s_red3[g][:], s_red3[g][:])
        nc.tensor.matmul(pn2[:], oselg[g][:], sq3[g][:], start=True, stop=True)
        a2 = float(alpha * alpha)
        g_ln = gp.tile([16, B], F32, tag="gln", name=f"gln{t}{g}")
        nc.scalar.activation(g_ln[:], pn2[:], AF.Ln, scale=a2)
        g_rt = gp.tile([16, B], F32, tag="grt", name=f"grt{t}{g}")
        nc.scalar.activation(g_rt[:], g_ln[:], AF.Exp, scale=0.5)
        g_d = gp.tile([16, B], F32, tag="gd", name=f"gd{t}{g}")
        nc.vector.tensor_scalar(g_d[:], pn2[:], float(alpha), 1.0 / float(alpha),
                                ALU.mult, ALU.add)
        g_r = gp.tile([16, B], F32, tag="gr", name=f"gr{t}{g}")
        nc.vector.reciprocal_approx_fast(g_r[:], g_d[:])
        g_hat = gp.tile([16, B], BF16, tag="ghat", name=f"ghat{t}{g}")
        nc.vector.tensor_mul(g_hat[:], g_rt[:], g_r[:])
        gd = dram.tile([16, B], BF16, tag="gdram", name=f"gd{t}{g}", bufs=2)
        nc.scalar.dma_start(gd[:], g_hat[:])
        for u in range(nu):
            o = 4 * g + u
            nc.scalar.dma_start(
                grep3[g][32 * u:32 * u + 16, :],
                gd[o:o + 1, :].broadcast_to((16, B)),
            )
        nc.vector.tensor_mul(sTg3[g][:], grep3[g][:], s_red3[g][:])

    zseed = {}
    # agreement unit policy per chunk: A = ACT copy + DVE 2x mul, D = direct
    POLICY = "ADADADADD"

    def agreement_g(t, g, nu):
        """b-logits (PSUM) -> e=exp(b) slices for o in group g."""
        pts = {}

        def emit_pt(u, c):
            pt = psp.tile([128, B], F32, tag="ps", name=f"t{t}{4 * g + u}{c}")
            nc.tensor.matmul(
                pt[:], w2tp[g][32 * u:32 * (u + 1), 128 * c:128 * (c + 1)],
                sTg3[g][32 * u:32 * (u + 1), :],
                start=True, stop=True, tile_position=(32 * u, 0))
            pts[(u, c)] = pt

        emit_pt(0, 0)
        for u in range(nu):
            o = 4 * g + u
            pba = psp.tile([128, B], F32, tag="ps", name=f"ba{t}{o}")
            pbb = psp.tile([16, B], F32, tag="ps", name=f"bb{t}{o}")
            if t == 1:
                nc.tensor.matmul(pba[:], ident[:], state_a[:, sl(o)],
                                 start=True, stop=False)
                nc.tensor.matmul(pbb[:], ident[0:16, 0:16], state_b[:, sl(o)],
                                 start=True, stop=False)
            for c in range(NCH):
                pt = pts.pop((u, c))
                # pipeline: next pt ahead of this chunk's dependent bd matmul
                if c + 1 < NCH:
                    emit_pt(u, c + 1)
                elif u + 1 < nu:
                    emit_pt(u + 1, 0)
                p = pp.tile([128, B], BF16, tag="p", name="p")
                if POLICY[c] == "A":
                    tsb = tsbp.tile([128, B], BF16, tag="tsb", name="tsb")
                    nc.scalar.copy(tsb[:], pt[:])
                    nc.vector.tensor_mul(p[:], tsb[:], xT(c))
                else:
                    nc.vector.tensor_mul(p[:], pt[:], xT(c))
                if c < 8:
                    nc.tensor.matmul(pba[:], bd[:, 128 * c:128 * (c + 1)],
                                     p[:], start=(t == 0 and c == 0), stop=(c == 7))
                else:
                    nc.tensor.matmul(pbb[:], bd[:, 0:16], p[:],
                                     start=(t == 0), stop=True)
            # e = exp(b-logits) straight from PSUM; persist state on iter 0
            nc.scalar.activation(e_a[:, sl(o)], pba[:], AF.Exp)
            nc.scalar.activation(e_b[:, sl(o)], pbb[:], AF.Exp)
            if t == 0:
                nc.vector.tensor_copy(state_a[:, sl(o)], pba[:])
                nc.vector.tensor_copy(state_b[:, sl(o)], pbb[:])
            za, zb = zseed[t]
            if o == 0:
                nc.vector.tensor_copy(za[:], e_a[:, sl(o)])
                nc.gpsimd.tensor_copy(zb[:], e_b[:, sl(o)])
            else:
                nc.vector.tensor_add(za[:], za[:], e_a[:, sl(o)])
                nc.gpsimd.tensor_add(zb[:], zb[:], e_b[:, sl(o)])

    def softmax_tail(t):
        """normalize e_a/e_b in place by 1/z; write c to DRAM."""
        za, zb = zseed[t]
        zaf = gp.tile([128, B], F32, tag="zaf", name=f"zaf{t}")
        zbf = gp.tile([16, B], F32, tag="zbf", name=f"zbf{t}")
        nc.vector.tensor_copy(zaf[:], za[:])
        nc.gpsimd.tensor_copy(zbf[:], zb[:])
        zira = gp.tile([128, B], F32, tag="zira", name=f"zira{t}")
        zirb = gp.tile([16, B], F32, tag="zirb", name=f"zirb{t}")
        nc.vector.reciprocal_approx_fast(zira[:], zaf[:])
        nc.vector.reciprocal_approx_fast(zirb[:], zbf[:])
        zi_a = gp.tile([128, B], BF16, tag="zia", name=f"zia{t}")
        zi_b = gp.tile([16, B], BF16, tag="zib", name=f"zib{t}")
        nc.vector.tensor_copy(zi_a[:], zira[:])
        nc.gpsimd.tensor_copy(zi_b[:], zirb[:])
        ea3 = e_a[:].rearrange("p (o b) -> p o b", o=N_OUT)
        eb3 = e_b[:].rearrange("p (o b) -> p o b", o=N_OUT)
        nc.vector.tensor_mul(
            ea3, ea3, zi_a[:].unsqueeze(1).broadcast_to((128, N_OUT, B)))
        nc.gpsimd.tensor_mul(
            eb3, eb3, zi_b[:].unsqueeze(1).broadcast_to((16, N_OUT, B)))
        nc.sync.dma_start(c_dram[t][0:128, :], e_a[:])
        nc.scalar.dma_start(c_dram[t][128:NLOC, :], e_b[:])

    def y_s_group(itn, g, nu):
        """s_part3[g] rows 32u:+16 <- sum_f W2[f,(o,:)] * (c (.) x)[f,:]."""
        w0 = 4 * g
        cd = c_dram[itn - 1]
        psos = psp.tile([128, B], F32, tag="ps", name=f"so{itn}{g}")
        for c in range(NCH):
            cx = cxp.tile([128, 4 * B], BF16, tag="cx", name="cx")
            (nc.sync if c % 2 else nc.scalar).dma_start(
                cx[:, 0:nu * B],
                cd[16 * c:16 * (c + 1),
                   B * w0:B * (w0 + nu)].unsqueeze(1).broadcast_to(
                    (16, 8, nu * B)),
            )
            y = yp.tile([128, 4 * B], BF16, tag="y", name="y")
            xbc = xT(c).unsqueeze(1).broadcast_to((128, nu, B))
            ybc = y[:, 0:nu * B].rearrange("p (o b) -> p o b", o=nu)
            cbc = cx[:, 0:nu * B].rearrange("p (o b) -> p o b", o=nu)
            nc.vector.tensor_mul(ybc, cbc, xbc)
            for u in range(nu):
                o = w0 + u
                nc.tensor.matmul(psos[32 * u:32 * (u + 1), :],
                                 w2p[c][:, 32 * o:32 * (o + 1)],
                                 y[:, B * u:B * (u + 1)],
                                 start=(c == 0), stop=(c == NCH - 1),
                                 tile_position=(0, 32 * u))
        nc.scalar.copy(s_part3[g][:], psos[:])

    # =====================  routing  =====================================
    for t in range(2):
        zseed[t] = (small.tile([128, B], BF16, tag=f"za{t}", name=f"za{t}"),
                    small.tile([16, B], BF16, tag=f"zb{t}", name=f"zb{t}"))
        alpha = 0.1 if t == 0 else 1.0
        for (g, nu) in GROUPS:
            for u in range(nu):
                o = 4 * g + u
                nc.sync.dma_start(s_red3[g][32 * u:32 * u + 16, :],
                                  ar_out[t][16 * o:16 * (o + 1), :])
            g_chain_g(t, g, nu, alpha)
            agreement_g(t, g, nu)
        softmax_tail(t)
        for (g, nu) in GROUPS:
            y_s_group(t + 1, g, nu)
            if t == 0:
                ship_block(1, g, nu, s_part3[g][:])
            else:
                for u in range(nu):
                    o = 4 * g + u
                    nc.sync.dma_start(out_d[16 * o:16 * (o + 1), :],
                                      s_part3[g][32 * u:32 * u + 16, :])

    ctx.close()


def _prep_inputs(x, weight):
    """Host-side layout prep. Returns per-core input maps."""
    x = np.asarray(x, dtype=np.float32)
    weight = np.asarray(weight, dtype=np.float32)
    bd_all = np.zeros((128, 8 * 128), dtype=bfnp)
    for cp in range(8):
        for p in range(128):
            bd_all[p, 128 * cp + 16 * cp + p // 8] = 1.0
    oselg = np.zeros((3, 128, 16), dtype=bfnp)
    for g in range(3):
        for u in range(4 if g < 2 else 2):
            oselg[g, 32 * u:32 * u + 16, 4 * g + u] = 1.0
    oselg = oselg.reshape(384, 16)
    ident = np.eye(128, dtype=bfnp)
    in_maps = []
    for k in range(NCORES):
        n0, n1 = NLOC * k, NLOC * (k + 1)
        xs = x[:, n0:n1, :]                      # [B, 144, 8]
        xT = np.ascontiguousarray(
            xs.transpose(1, 2, 0).reshape(F, B)).astype(bfnp)
        Wk = weight[:, n0:n1, :, :]              # [10, 144, 16, 8]
        w2 = np.ascontiguousarray(
            Wk.transpose(1, 3, 0, 2).reshape(F, OI)).astype(bfnp)
        w2t = np.ascontiguousarray(w2.T)          # [160, F]
        w2tp = np.zeros((3, 128, F), dtype=bfnp)
        for g in range(3):
            for u in range(4 if g < 2 else 2):
                o = 4 * g + u
                w2tp[g, 32 * u:32 * u + 16, :] = w2t[16 * o:16 * (o + 1), :]
        w2tp = w2tp.reshape(384, F)
        w2p = np.zeros((F, 320), dtype=bfnp)
        for o in range(N_OUT):
            w2p[:, 32 * o:32 * o + 16] = w2[:, 16 * o:16 * (o + 1)]
        in_maps.append({
            "xT": xT, "w2": w2, "w2t": w2tp,
            "w2p": w2p, "bd": bd_all, "osel": oselg, "ident": ident,
        })
    return in_maps


def _squash_np(s):
    norm = np.linalg.norm(s, axis=-1, keepdims=True)
    return (norm ** 2 / (1.0 + norm ** 2) / (norm + 1e-8)) * s


def run_spmd(x, weight, trace=False, tmpdir=None):
    global _built
    if _built is None:
        _built = _build()
    nc = _built
    in_maps = _prep_inputs(x, weight)
    res = run_bass_kernel_spmd(
        nc, in_maps, list(range(NCORES)), trace=trace, tmpdir=tmpdir)
    s2 = np.zeros((OI, B), dtype=np.float32)
    for k in range(NCORES):
        s2 += res.results[k]["out"].astype(np.float32)
    s2 = s2.reshape(N_OUT, D_OUT, B).transpose(2, 0, 1)  # [B, 10, 16]
    out = _squash_np(s2).astype(np.float32)
    return out, res


def kernel(x, weight):
    out, _ = run_spmd(x, weight)
    return out
